# revision 13
# baseline (speedup 1.0000x reference)
"""Trainium2 Bass kernel for nn_EncoderLayer_2035814498815 (sparse_attention).

Sharding: 8 cores = (batch sample b in 0..3) x (query half in 0..1).
Zero collectives: each core computes 512 query rows of sample b through the
whole layer (5 MHA branches + gating + LN1 + FFN + LN2); host concatenates.

One SPMD program for all 8 cores: the host rotates the KEY order of every
KV source (and the masks) by 512*half per core, which makes the 7x7-window
block-sparsity geometry identical across cores (attention is permutation-
invariant over keys). Queries always live at rotated columns [0, 512).

On-core layout is transposed ([D, seq]):
- scores come out as ST = K @ Q^T  [Sk_p, Sq_f] (contraction over head dim)
- exp needs no max-subtraction (|scores| ~ 0.2; 1/sqrt(hd) folded into wq)
- A@V uses V as stationary operand augmented with a ones column per head,
  so PSUM row 64 accumulates softmax denominators for free
- denominators are gathered per unit into an [8, 512] tile by tiny DMAs,
  one reciprocal_approx_fast serves all heads; gate/rowsum fused via 1/gate
- out-projection of all units accumulates into one new_x tile; the
  out-proj bias + V-bias terms enter as a host-precomputed rank-5 matmul
  OB^T @ gate^T; LN affine folds into fc1 (g1,b1) and the host (g2,b2).

All matmuls run as float32r (~1.6e-4 rel err, full PE rate at N>=256).
"""

import sys

sys.path.insert(0, "/opt/trn_rl_repo")

import numpy as np
import ml_dtypes

import concourse.bass as bass
import concourse.mybir as mybir
import concourse.tile as tile
from concourse.bass_utils import run_bass_kernel_spmd

F32 = mybir.dt.float32
F32R = mybir.dt.float32r
BF16 = mybir.dt.bfloat16
ACT = mybir.ActivationFunctionType
ALU = mybir.AluOpType

B, S, D, H, HD, F, REF = 4, 1024, 512, 8, 64, 2048, 2
HGRID, HALF = 32, 3
SQ = 512
NDC = D // 128     # 4
NJT = S // 128     # 8
NFT = F // 128     # 16
BRANCHES = ["mca", "ca", "msa", "nsa", "sa"]

# window of local query columns whose 7x7 grid window can intersect rotated
# key tile j  (rotated coords: local i1' in [0,16), key j1' = 4j..4j+3)
def _window(j):
    if j <= 4:
        return (max(0, 4 * j - 3) * 32, min(16, 4 * j + 7) * 32)
    if j == 7:
        return (0, 96)          # wrap-around block (real only on half==1)
    return None


MCA_JS = [j for j in range(NJT) if _window(j) is not None]


def build_nc():
    nc = bass.Bass(trn_type="TRN2")
    dram = {}

    def din(name, shape, dt=F32):
        dram[name] = nc.dram_tensor(name, shape, dt, kind="ExternalInput")

    din("xT", [D, S])
    din("refT", [D, REF * S])
    for p in BRANCHES:
        din(f"w_{p}", [D, 3 * D])
        din(f"ow_{p}", [64, 8 * D])
        din(f"bqk_{p}", [128, 2 * NDC])
    din("gateT", [8, SQ])
    din("ginv8", [8, 5 * SQ])
    din("expand8", [8, 8 * HD])
    din("ones128", [128, 8])
    din("ones512", [1, SQ])
    din("zeros512", [128, SQ])
    din("OB", [8, D])
    din("mrevT", [S, SQ], BF16)
    din("mfwdT", [S, SQ], BF16)
    din("fc1T", [D, F])
    din("fc1b", [128, NFT])
    din("fc2T", [F, D])
    din("fc2b", [8, D])
    din("g1b1", [128, 2 * NDC])
    out_t = nc.dram_tensor("z2T", [D, SQ], F32, kind="ExternalOutput")

    with tile.TileContext(nc) as tc:
        _emit(nc, tc, dram, out_t)
    return nc


def _emit(nc, tc, dram, out_t):
    def r128(name):
        return dram[name].rearrange("(c p) f -> p c f", p=128)

    # ---------------- global pools (live whole kernel) ----------------
    with tc.tile_pool(name="glob", bufs=1) as gp, \
         tc.tile_pool(name="globps", bufs=1, space="PSUM") as gpps:

        nx = gp.tile([128, NDC, SQ], F32R, tag="nx")
        ones8 = gp.tile([128, 8], F32R, tag="ones8")
        nc.sync.dma_start(ones8[:], dram["ones128"][:].bitcast(F32R))
        ones_col = ones8
        ones_row = gp.tile([1, SQ], F32R, tag="ones_row")
        nc.sync.dma_start(ones_row[:], dram["ones512"][:].bitcast(F32R))
        zero512 = gp.tile([128, SQ], F32R, tag="zero512")
        nc.sync.dma_start(zero512[:], dram["zeros512"][:].bitcast(F32R))
        eps = gp.tile([1, 1], F32, tag="eps")
        nc.vector.memset(eps[:], 1e-5)
        gateT = gp.tile([8, SQ], F32R, tag="gateT")
        nc.sync.dma_start(gateT[:], dram["gateT"][:].bitcast(F32R))
        ginv8 = gp.tile([8, 5 * SQ], F32, tag="ginv8")
        nc.sync.dma_start(ginv8[:], dram["ginv8"][:])
        expand8 = gp.tile([8, 8 * HD], F32R, tag="expand8")
        nc.sync.dma_start(expand8[:], dram["expand8"][:].bitcast(F32R))
        OB = gp.tile([8, D], F32R, tag="OB")
        nc.sync.dma_start(OB[:], dram["OB"][:].bitcast(F32R))
        g1b1 = gp.tile([128, 2 * NDC], F32, tag="g1b1")
        nc.sync.dma_start(g1b1[:], dram["g1b1"][:])
        fc1b = gp.tile([128, NFT], F32, tag="fc1b")
        nc.sync.dma_start(fc1b[:], dram["fc1b"][:])
        fc2b = gp.tile([8, D], F32R, tag="fc2b")
        nc.sync.dma_start(fc2b[:], dram["fc2b"][:].bitcast(F32R))
        bqk = {}
        for p in BRANCHES:
            t = gp.tile([128, 2 * NDC], F32, tag=f"bqk_{p}")
            nc.sync.dma_start(t[:], dram[f"bqk_{p}"][:])
            bqk[p] = t

        _attention(nc, tc, dram, r128, nx, zero512, ones8, gateT, ginv8,
                   expand8, OB, bqk)
        _ffn(nc, tc, dram, r128, out_t, nx, ones_col, ones_row, eps, g1b1,
             fc1b, fc2b, gpps)


def _attention(nc, tc, dram, r128, nx, zero512, ones8, gateT, ginv8,
                   expand8, OB, bqk):
    with tc.tile_pool(name="att", bufs=1) as ap, \
         tc.tile_pool(name="att2", bufs=2) as ap2, \
         tc.tile_pool(name="attoct", bufs=1) as octp, \
         tc.tile_pool(name="attpt", bufs=3) as ptp, \
         tc.tile_pool(name="attsm", bufs=1) as smp, \
         tc.tile_pool(name="attps", bufs=2, space="PSUM") as psg, \
         tc.tile_pool(name="attps1", bufs=1, space="PSUM") as ps1:

        mrev = ap.tile([128, NJT, SQ], BF16, tag="mrev")
        nc.sync.dma_start(mrev[:], dram["mrevT"].rearrange("(j p) q -> p j q", p=128)[:])
        mfwd = ap.tile([128, NJT, SQ], BF16, tag="mfwd")
        nc.sync.dma_start(mfwd[:], dram["mfwdT"].rearrange("(j p) q -> p j q", p=128)[:])
        xT = ap.tile([128, NDC, S], F32R, tag="xT")
        nc.sync.dma_start(xT[:], r128("xT")[:].bitcast(F32R))

        # (branch, kv source name, col offset, mask, K-src override)
        units = [
            ("mca", "refT", 0, "rev"),
            ("mca", "refT", S, "rev"),
            ("ca", "refT", S, None),
            ("nsa", "refT", S, "fwd"),   # K from ref_last, V from x
            ("msa", "xT", 0, "fwd"),
            ("sa", "xT", 0, None),
        ]

        first_op = [True]
        qt_mca = [None]

        for uidx, (p, srcname, coff, mask) in enumerate(units):
            w_sb = ap.tile([128, NDC, 3 * D], F32R, tag="w")
            nc.sync.dma_start(w_sb[:], r128(f"w_{p}")[:].bitcast(F32R))
            if srcname == "xT":
                ksrc = xT
                kcoff = 0
            else:
                ksrc = ap2.tile([128, NDC, S], F32R, tag="kvsrc")
                nc.sync.dma_start(
                    ksrc[:],
                    dram["refT"].rearrange("(c p) f -> p c f", p=128)
                    [:, :, coff:coff + S].bitcast(F32R))
                kcoff = 0
            vsrc, vcoff = (xT, 0) if p in ("nsa", "msa", "sa") else (ksrc, kcoff)

            # ---- projections ----
            if p == "mca" and qt_mca[0] is not None:
                qt = qt_mca[0]
            else:
                qt = ap.tile([128, NDC, SQ], F32R, tag="qt")
                for t in range(NDC):
                    ps = psg.tile([128, SQ], F32, tag="psgen")
                    for c in range(NDC):
                        nc.tensor.matmul(
                            ps[:], w_sb[:, c, 128 * t:128 * (t + 1)],
                            xT[:, c, 0:SQ], start=(c == 0), stop=(c == NDC - 1))
                    nc.vector.tensor_scalar(qt[:, t], ps[:], bqk[p][:, t:t + 1],
                                            None, ALU.add)
                if p == "mca":
                    qt_mca[0] = qt
            kt = ap.tile([128, NDC, S], F32R, tag="kt")
            for t in range(NDC):
                for s in range(2):
                    ps = psg.tile([128, SQ], F32, tag="psgen")
                    for c in range(NDC):
                        nc.tensor.matmul(
                            ps[:], w_sb[:, c, D + 128 * t:D + 128 * (t + 1)],
                            ksrc[:, c, kcoff + SQ * s:kcoff + SQ * (s + 1)],
                            start=(c == 0), stop=(c == NDC - 1))
                    nc.vector.tensor_scalar(
                        kt[:, t, SQ * s:SQ * (s + 1)], ps[:],
                        bqk[p][:, NDC + t:NDC + t + 1], None, ALU.add)
            js = MCA_JS if mask == "rev" else list(range(NJT))
            va = ap.tile([128, NJT, H, HD + 1], F32R, tag="va")
            for j in js:
                ps = psg.tile([128, SQ], F32, tag="psgen")
                for c in range(NDC):
                    nc.tensor.matmul(
                        ps[:], vsrc[:, c, vcoff + 128 * j:vcoff + 128 * (j + 1)],
                        w_sb[:, c, 2 * D:3 * D],
                        start=(c == 0), stop=(c == NDC - 1))
                nc.vector.tensor_copy(
                    va[:, j, :, 0:HD], ps[:].rearrange("p (h d) -> p h d", h=H))
                nc.vector.tensor_copy(va[:, j, :, HD:HD + 1],
                                      ones8[:, 0:H].unsqueeze(2))

            # ---- attention ----
            rs = smp.tile([H, SQ], F32, tag="rs")
            oct_sb = octp.tile([65, H, SQ], F32R, tag="oct")
            for hp in range(H // 2):
                av2 = ps1.tile([128, 2, SQ], F32, tag="av2")
                for hh in range(2):
                    h = 2 * hp + hh
                    t, r0 = h // 2, 64 * (h % 2)
                    first = True
                    for j in js:
                        win = _window(j) if mask else None
                        if mask == "rev":
                            qlo, qhi = win
                        else:
                            qlo, qhi = 0, SQ
                        stps = psg.tile([128, SQ], F32, tag="stps")
                        ptt = ptp.tile([128, SQ], F32R, tag="ptt")
                        if mask == "rev":
                            nc.vector.tensor_copy(ptt[:], zero512[:])
                        nc.tensor.matmul(
                            stps[:, qlo:qhi],
                            kt[r0:r0 + 64, t, 128 * j:128 * (j + 1)],
                            qt[r0:r0 + 64, t, qlo:qhi],
                            start=True, stop=True)
                        nc.scalar.activation(ptt[:, qlo:qhi], stps[:, qlo:qhi],
                                             ACT.Exp)
                        if mask == "rev":
                            nc.vector.tensor_tensor(
                                ptt[:, qlo:qhi], ptt[:, qlo:qhi].bitcast(F32),
                                mrev[:, j, qlo:qhi], ALU.mult)
                        elif mask == "fwd" and win is not None:
                            wl, wh = win
                            nc.vector.tensor_tensor(
                                ptt[:, wl:wh], ptt[:, wl:wh].bitcast(F32),
                                mfwd[:, j, wl:wh], ALU.mult)
                        nc.tensor.matmul(
                            av2[0:HD + 1, hh, :], va[:, j, h, :], ptt[:],
                            start=first, stop=(j == js[-1]))
                        first = False
                    nc.vector.tensor_copy(oct_sb[64:65, h, :],
                                          av2[HD:HD + 1, hh, :])
                nc.vector.tensor_copy(oct_sb[0:HD, 2 * hp, :], av2[0:HD, 0, :])
                nc.vector.tensor_copy(oct_sb[0:HD, 2 * hp + 1, :], av2[0:HD, 1, :])
            # gather denominators (lane 64 -> lanes 0..7) and normalize
            nc.sync.dma_start(rs[:], oct_sb[64:65, :, :].bitcast(F32))
            grinv = smp.tile([H, SQ], F32, tag="grinv")
            gr = BRANCHES.index(p)
            nc.vector.tensor_tensor(
                grinv[:], rs[:], ginv8[:, gr * SQ:(gr + 1) * SQ], ALU.mult)
            nc.vector.reciprocal(grinv[:], grinv[:])
            grinvr = smp.tile([H, SQ], F32R, tag="grinvr")
            nc.vector.tensor_copy(grinvr[:], grinv[:])
            for h in range(H):
                gx = psg.tile([128, SQ], F32, tag="stps")
                nc.tensor.matmul(gx[0:HD, :], expand8[:, h * HD:(h + 1) * HD],
                                 grinvr[:], start=True, stop=True)
                nc.vector.tensor_tensor(
                    oct_sb[0:HD, h, :],
                    oct_sb[0:HD, h, :].bitcast(F32),
                    gx[0:HD, :], ALU.mult)
            # out-projection accumulate into nx
            ow_sb = ap.tile([64, H, D], F32R, tag="ow")
            nc.sync.dma_start(
                ow_sb[:], dram[f"ow_{p}"].rearrange("p (h d) -> p h d", h=H)[:].bitcast(F32R))
            last_u = uidx == len(units) - 1
            for t in range(NDC):
                ps = ps1.tile([128, SQ], F32, tag="psop")
                for hc in range(H):
                    nc.tensor.matmul(
                        ps[:], ow_sb[:, hc, 128 * t:128 * (t + 1)],
                        oct_sb[0:HD, hc, :],
                        start=(hc == 0), stop=(hc == H - 1 and not last_u))
                if last_u:
                    nc.tensor.matmul(
                        ps[:], OB[0:5, 128 * t:128 * (t + 1)], gateT[0:5, :],
                        start=False, stop=True)
                if first_op[0]:
                    nc.vector.tensor_copy(nx[:, t], ps[:])
                else:
                    nc.vector.tensor_tensor(nx[:, t], nx[:, t].bitcast(F32), ps[:],
                                            ALU.add)
            first_op[0] = False


def _layernorm(nc, lnp, psg, ones_col, ones_row, eps, src, dst):
    """dst[:, c] = (src[:, c] - mean_D) / sqrt(var_D + eps); src F32R [NDC,128,SQ]."""
    stats = psg.tile([128, SQ], F32, tag="psgen")
    stats2 = psg.tile([128, SQ], F32, tag="psgen")
    sq = lnp.tile([128, NDC, SQ], F32R, tag="sq")
    for c in range(NDC):
        nc.scalar.activation(sq[:, c], src[:, c].bitcast(F32), ACT.Square)
    for c in range(NDC):
        nc.tensor.matmul(stats[0:1, :], ones_col[:, 0:1], src[:, c],
                         start=(c == 0), stop=(c == NDC - 1))
    for c in range(NDC):
        nc.tensor.matmul(stats2[0:1, :], ones_col[:, 0:1], sq[:, c],
                         start=(c == 0), stop=(c == NDC - 1))
    sc = lnp.tile([1, 4 * SQ], F32, tag="lnsc")   # mean | msq | var | rstd
    nc.vector.tensor_scalar(sc[0:1, 0:SQ], stats[0:1, :], 1.0 / D, None, ALU.mult)
    nc.vector.tensor_scalar(sc[0:1, SQ:2 * SQ], stats2[0:1, :], 1.0 / D, None,
                            ALU.mult)
    nc.vector.tensor_tensor(sc[0:1, 2 * SQ:3 * SQ], sc[0:1, 0:SQ],
                            sc[0:1, 0:SQ], ALU.mult)
    nc.vector.tensor_tensor(sc[0:1, 2 * SQ:3 * SQ], sc[0:1, SQ:2 * SQ],
                            sc[0:1, 2 * SQ:3 * SQ], ALU.subtract)
    nc.scalar.activation(sc[0:1, 3 * SQ:4 * SQ], sc[0:1, 2 * SQ:3 * SQ],
                         ACT.Sqrt, bias=eps[0:1, 0:1])
    nc.vector.reciprocal(sc[0:1, 3 * SQ:4 * SQ], sc[0:1, 3 * SQ:4 * SQ])
    scr = lnp.tile([1, 2 * SQ], F32R, tag="lnscr")
    nc.vector.tensor_copy(scr[0:1, 0:SQ], sc[0:1, 0:SQ])
    nc.vector.tensor_copy(scr[0:1, SQ:2 * SQ], sc[0:1, 3 * SQ:4 * SQ])
    meanx = psg.tile([128, SQ], F32, tag="psgen")
    rstdx = psg.tile([128, SQ], F32, tag="psgen")
    nc.tensor.matmul(meanx[:], ones_row[0:1, 0:128], scr[0:1, 0:SQ],
                     start=True, stop=True)
    nc.tensor.matmul(rstdx[:], ones_row[0:1, 0:128], scr[0:1, SQ:2 * SQ],
                     start=True, stop=True)
    for c in range(NDC):
        t = lnp.tile([128, SQ], F32, tag="lntmp")
        nc.vector.tensor_tensor(t[:], src[:, c].bitcast(F32), meanx[:],
                                ALU.subtract)
        nc.vector.tensor_tensor(dst[:, c], t[:], rstdx[:], ALU.mult)


def _ffn(nc, tc, dram, r128, out_t, nx, ones_col, ones_row, eps, g1b1,
         fc1b, fc2b, gpps):
    with tc.tile_pool(name="ffn", bufs=1) as fp, \
         tc.tile_pool(name="ffnps", bufs=2, space="PSUM") as psg:
        z1 = fp.tile([128, NDC, SQ], F32R, tag="z1")
        _layernorm(nc, fp, psg, ones_col, ones_row, eps, nx, z1)

        fc1 = fp.tile([128, NDC, F], F32R, tag="fc1")
        nc.sync.dma_start(fc1[:], r128("fc1T")[:].bitcast(F32R))
        yT = fp.tile([128, NFT, SQ], F32R, tag="yT")
        for f in range(NFT):
            ps = psg.tile([128, SQ], F32, tag="psffn")
            for c in range(NDC):
                nc.tensor.matmul(ps[:], fc1[:, c, 128 * f:128 * (f + 1)], z1[:, c],
                                 start=(c == 0), stop=(c == NDC - 1))
            nc.scalar.activation(yT[:, f], ps[:], ACT.Gelu, bias=fc1b[:, f:f + 1])
        fc2 = fp.tile([128, NFT, D], F32R, tag="fc2")
        nc.sync.dma_start(fc2[:], dram["fc2T"].rearrange("(c p) d -> p c d", p=128)[:].bitcast(F32R))
        resid = fp.tile([128, NDC, SQ], F32R, tag="resid")
        for t in range(NDC):
            ps = psg.tile([128, SQ], F32, tag="psffn")
            for f in range(NFT):
                nc.tensor.matmul(ps[:], fc2[:, f, 128 * t:128 * (t + 1)], yT[:, f],
                                 start=(f == 0), stop=False)
            nc.tensor.matmul(ps[:], fc2b[0:1, 128 * t:128 * (t + 1)],
                             ones_row[:], start=False, stop=True)
            ht = fp.tile([128, SQ], F32, tag="ht")
            nc.vector.tensor_scalar(ht[:], z1[:, t].bitcast(F32), g1b1[:, t:t + 1],
                                    g1b1[:, NDC + t:NDC + t + 1], ALU.mult,
                                    ALU.add)
            nc.vector.tensor_tensor(resid[:, t], ht[:], ps[:], ALU.add)
        z2 = fp.tile([128, NDC, SQ], F32, tag="z2")
        _layernorm(nc, fp, psg, ones_col, ones_row, eps, resid, z2)
        for c in range(NDC):
            nc.sync.dma_start(out_t[128 * c:128 * (c + 1), :], z2[:, c])


# ---------------------------------------------------------------------------
def _split_excess_waits(nc):
    """Walrus caps sync waits (1/inst, 2 on EventSemaphore); peel extras
    onto NoOps inserted before the instruction on the same engine queue."""
    n = 0
    for f in nc.m.functions:
        for bb in f.blocks:
            new = []
            for inst in bb.instructions:
                si = inst.sync_info
                cap = 2 if isinstance(inst, mybir.InstEventSemaphore) else 1
                waits = list(si.on_wait) if si and si.on_wait else []
                if len(waits) > cap:
                    excess, keep = waits[:-cap], waits[-cap:]
                    for i, w in enumerate(excess):
                        nop = mybir.InstNoOp(name=f"{inst.name}_wsplit_{i}",
                                             ins=[], outs=[])
                        nop.engine = inst.engine
                        nop.sync_info = mybir.SyncInfo(on_wait=[w], on_update=[])
                        new.append(nop)
                        n += 1
                    si.on_wait = keep
                    inst.sync_info = si
                new.append(inst)
            bb.instructions = new
    return n


# ---------------------------------------------------------------------------
def _host_prep(inputs):
    x = np.asarray(inputs["x"], np.float32)
    ref = np.asarray(inputs["ref_mca"], np.float32)
    gate = np.asarray(inputs["gate"], np.float32)

    i = np.arange(HGRID)
    near = np.abs(i[:, None] - i[None, :]) <= HALF
    inside = (near[:, None, :, None] & near[None, :, None, :]).reshape(S, S)

    def chunked(v):
        return np.ascontiguousarray(v.reshape(NDC, 128).T)

    per_branch = {}
    OB = np.zeros((8, D), np.float32)
    for bi, p in enumerate(BRANCHES):
        w = np.asarray(inputs[p + "_w"], np.float32)
        b = np.asarray(inputs[p + "_b"], np.float32)
        ow = np.asarray(inputs[p + "_ow"], np.float32)
        ob = np.asarray(inputs[p + "_ob"], np.float32)
        sc = 1.0 / np.sqrt(np.float32(HD))
        wq, wk, wv = w[:D] * sc, w[D:2 * D], w[2 * D:]
        bq, bk, bv = b[:D] * sc, b[D:2 * D], b[2 * D:]
        wT = np.ascontiguousarray(np.concatenate([wq, wk, wv], 0).T)
        bqkv = np.concatenate([chunked(bq), chunked(bk)], 1)
        mult = 2.0 if p == "mca" else 1.0
        OB[bi] = mult * ob + mult * (ow @ bv)
        ow8 = np.ascontiguousarray(
            ow.T.reshape(H, HD, D).transpose(1, 0, 2).reshape(HD, H * D))
        per_branch[p] = (wT, ow8, bqkv)

    g1 = np.asarray(inputs["ln1_g"], np.float32)
    b1 = np.asarray(inputs["ln1_b"], np.float32)
    fc1 = np.asarray(inputs["fc1_w"], np.float32)
    fc1_b = np.asarray(inputs["fc1_b"], np.float32)
    fc2 = np.asarray(inputs["fc2_w"], np.float32)
    fc2_b = np.asarray(inputs["fc2_b"], np.float32)
    fc1T = np.ascontiguousarray((fc1 * g1[None, :]).T)
    fc1bv = np.ascontiguousarray((fc1_b + fc1 @ b1).reshape(NFT, 128).T)
    fc2T = np.ascontiguousarray(fc2.T)
    fc2bv = np.zeros((8, D), np.float32)
    fc2bv[0] = fc2_b
    g1b1 = np.concatenate([chunked(g1), chunked(b1)], 1)

    expand8 = np.zeros((8, 8 * HD), np.float32)
    for h in range(8):
        expand8[h, h * HD:(h + 1) * HD] = 1.0
    in_maps = []
    for core in range(8):
        b_, half = core // 2, core % 2
        q0 = half * SQ
        roll = -q0
        xTr = np.roll(x[b_].T, roll, axis=1)
        refTr = np.concatenate(
            [np.roll(ref[b_, r * S:(r + 1) * S].T, roll, axis=1)
             for r in range(REF)], axis=1)
        insT = np.roll(inside[q0:q0 + SQ, :].T, roll, axis=0)
        mrevT = insT.astype(ml_dtypes.bfloat16)
        mfwdT = (1.0 - insT).astype(ml_dtypes.bfloat16)
        m = {
            "xT": np.ascontiguousarray(xTr),
            "refT": np.ascontiguousarray(refTr),
            "gateT": np.zeros((8, SQ), np.float32),
            "ginvT": np.zeros((8, SQ), np.float32),
            "OB": OB, "mrevT": mrevT, "mfwdT": mfwdT,
            "fc1T": fc1T, "fc1b": fc1bv, "fc2T": fc2T, "fc2b": fc2bv,
            "g1b1": g1b1,
        }
        gslice = gate[b_, q0:q0 + SQ, :].T
        m["gateT"][:5] = gslice
        gi = 1.0 / np.maximum(gslice, 1e-30)          # [5, SQ]
        m["ginv8"] = np.ascontiguousarray(
            np.repeat(gi[None, :, :], 8, axis=0).reshape(8, 5 * SQ))
        m["expand8"] = expand8
        m["ones128"] = np.ones((128, 8), np.float32)
        m["ones512"] = np.ones((1, SQ), np.float32)
        m["zeros512"] = np.zeros((128, SQ), np.float32)
        for p in BRANCHES:
            wT, owT, bqkv = per_branch[p]
            m[f"w_{p}"], m[f"ow_{p}"], m[f"bqk_{p}"] = wT, owT, bqkv
        in_maps.append(m)
    return in_maps


_cache = {}


def _get_nc():
    if "nc" not in _cache:
        nc = build_nc()
        _split_excess_waits(nc)
        _cache["nc"] = nc
    return _cache["nc"]


def _get_runner():
    """Compile once; return (fn(in_maps) -> per-core outs, in_names)."""
    if "runner" in _cache:
        return _cache["runner"]
    import jax
    from jax.sharding import Mesh, PartitionSpec
    from jax.experimental.shard_map import shard_map
    import concourse.mybir as mybir_
    from concourse import bass2jax

    nc = _get_nc()
    bass2jax.install_neuronx_cc_hook()
    in_names, out_names, out_avals = [], [], []
    pname = nc.partition_id_tensor.name if nc.partition_id_tensor else None
    for alloc in nc.m.functions[0].allocations:
        if not isinstance(alloc, mybir_.MemoryLocationSet):
            continue
        name = alloc.memorylocations[0].name
        if alloc.kind == "ExternalInput":
            if name != pname:
                in_names.append(name)
        elif alloc.kind == "ExternalOutput":
            out_names.append(name)
            out_avals.append(jax.core.ShapedArray(
                tuple(alloc.tensor_shape), mybir_.dt.np(alloc.dtype)))
    n_params = len(in_names)
    all_names = in_names + out_names + ([pname] if pname else [])

    def _body(*args):
        operands = list(args)
        if pname is not None:
            operands.append(bass2jax.partition_id_tensor())
        return tuple(bass2jax._bass_exec_p.bind(
            *operands, out_avals=tuple(out_avals), in_names=tuple(all_names),
            out_names=tuple(out_names), lowering_input_output_aliases=(),
            sim_require_finite=True, sim_require_nnan=True, nc=nc))

    devices = jax.devices()[:8]
    mesh = Mesh(np.asarray(devices), ("core",))
    nz = len(out_names)
    sharded = jax.jit(shard_map(
        _body, mesh=mesh,
        in_specs=(PartitionSpec("core"),) * (n_params + nz),
        out_specs=(PartitionSpec("core"),) * nz,
        check_rep=False), keep_unused=True)
    zero_shapes = [(8 * a.shape[0], *a.shape[1:]) for a in out_avals]
    zero_dtypes = [a.dtype for a in out_avals]

    def run(in_maps):
        concat_in = [np.concatenate([m[n] for m in in_maps], axis=0)
                     for n in in_names]
        zeros = [np.zeros(s, d) for s, d in zip(zero_shapes, zero_dtypes)]
        outs = sharded(*concat_in, *zeros)
        outs = [np.asarray(o) for o in outs]
        return [
            {n: outs[i].reshape(8, *out_avals[i].shape)[c]
             for i, n in enumerate(out_names)}
            for c in range(8)
        ]

    _cache["runner"] = (run, in_names, sharded, out_avals, out_names)
    return _cache["runner"]


def kernel(**inputs):
    in_maps = _host_prep(inputs)
    run = _get_runner()[0]
    results = run(in_maps)
    g2 = np.asarray(inputs["ln2_g"], np.float32)
    b2 = np.asarray(inputs["ln2_b"], np.float32)
    out = np.empty((B, S, D), np.float32)
    for core in range(8):
        b_, half = core // 2, core % 2
        out[b_, half * SQ:(half + 1) * SQ] = results[core]["z2T"].T
    return (out * g2[None, None, :] + b2[None, None, :]).astype(np.float32)


if __name__ == "__main__":
    nc = build_nc()
    n_inst = sum(len(bb.instructions) for f in nc.m.functions for bb in f.blocks)
    print("built ok, insts:", n_inst)
    print("wait splits:", _split_excess_waits(nc))


# revision 30
# speedup vs baseline: 25.2805x; 25.2805x over previous
"""Trainium2 Bass kernel for nn_EncoderLayer_2035814498815 (sparse_attention).

Sharding: 8 cores = (batch sample b in 0..3) x (query half in 0..1).
Zero collectives: each core computes 512 query rows of sample b through the
whole layer (5 MHA branches + gating + LN1 + FFN + LN2); host concatenates.

One SPMD program for all 8 cores: the host rotates the KEY order of every
KV source (and the masks) by 512*half per core, which makes the 7x7-window
block-sparsity geometry identical across cores (attention is permutation-
invariant over keys). Queries always live at rotated columns [0, 512).

On-core layout is transposed ([D, seq]):
- scores come out as ST = K @ Q^T  [Sk_p, Sq_f] (contraction over head dim)
- exp needs no max-subtraction (|scores| ~ 0.2; 1/sqrt(hd) folded into wq)
- A@V uses V as stationary operand augmented with a ones column per head,
  so PSUM row 64 accumulates softmax denominators for free
- denominators are gathered per unit into an [8, 512] tile by tiny DMAs,
  reciprocals run as Ln->Exp(-x) on ACT (same table set as the attention
  exp); gate/rowsum fused via host-sent 1/gate
- out-projection of all units accumulates into one new_x tile; the
  out-proj bias + V-bias terms enter as a host-precomputed rank-5 matmul
  OB^T @ gate^T; LN affine folds into fc1 (g1,b1) and the host (g2,b2).

All matmuls run as float32r (~1.6e-4 rel err, full PE rate at N>=256).
"""

import sys

sys.path.insert(0, "/opt/trn_rl_repo")

import numpy as np
import ml_dtypes

import concourse.bass as bass
import concourse.mybir as mybir
import concourse.tile as tile
from concourse.bass_utils import run_bass_kernel_spmd

F32 = mybir.dt.float32
F32R = mybir.dt.float32r
BF16 = mybir.dt.bfloat16
ACT = mybir.ActivationFunctionType
ALU = mybir.AluOpType

B, S, D, H, HD, F, REF = 4, 1024, 512, 8, 64, 2048, 2
HGRID, HALF = 32, 3
SQ = 512
NDC = D // 128     # 4
NJT = S // 128     # 8
NFT = F // 128     # 16
BRANCHES = ["mca", "ca", "msa", "nsa", "sa"]

# window of local query columns whose 7x7 grid window can intersect rotated
# key tile j  (rotated coords: local i1' in [0,16), key j1' = 4j..4j+3)
def _window(j):
    if j <= 4:
        return (max(0, 4 * j - 3) * 32, min(16, 4 * j + 7) * 32)
    if j == 7:
        return (0, 96)          # wrap-around block (real only on half==1)
    return None


MCA_JS = [j for j in range(NJT) if _window(j) is not None]


def build_nc():
    nc = bass.Bass(trn_type="TRN2")
    dram = {}

    def din(name, shape, dt=F32):
        dram[name] = nc.dram_tensor(name, shape, dt, kind="ExternalInput")

    din("xT", [D, S])
    din("refT", [D, REF * S])
    for p in BRANCHES:
        din(f"w_{p}", [D, 3 * D])
        din(f"ow_{p}", [64, 8 * D])
        din(f"bqk_{p}", [128, 2 * NDC])
    din("gateT", [8, SQ])
    din("ginv8", [8, 5 * SQ])
    din("expand8", [8, 8 * HD])
    din("ones128", [128, 8])
    din("ones512", [1, SQ])
    din("OB", [8, D])
    din("mrevT", [S, SQ], BF16)
    din("mfwdT", [S, SQ], BF16)
    din("fc1T", [D, F])
    din("fc1b", [128, NFT])
    din("fc2T", [F, D])
    din("fc2b", [8, D])
    din("g1b1", [128, 2 * NDC])
    out_t = nc.dram_tensor("z2T", [D, SQ], F32, kind="ExternalOutput")

    with tile.TileContext(nc) as tc:
        _emit(nc, tc, dram, out_t)
    return nc


def _emit(nc, tc, dram, out_t):
    def r128(name):
        return dram[name].rearrange("(c p) f -> p c f", p=128)

    # ---------------- global pools (live whole kernel) ----------------
    with tc.tile_pool(name="glob", bufs=1) as gp, \
         tc.tile_pool(name="globps", bufs=1, space="PSUM") as gpps:

        nx = gp.tile([128, NDC, SQ], F32R, tag="nx")
        ones8 = gp.tile([128, 8], F32R, tag="ones8")
        ones_col = ones8
        ones_row = gp.tile([1, SQ], F32R, tag="ones_row")
        eps = gp.tile([1, 1], F32, tag="eps")
        gateT = gp.tile([8, SQ], F32R, tag="gateT")
        ginv8 = gp.tile([8, 5 * SQ], F32, tag="ginv8")
        expand8 = gp.tile([8, 8 * HD], F32R, tag="expand8")
        OB = gp.tile([8, D], F32R, tag="OB")
        g1b1 = gp.tile([128, 2 * NDC], F32, tag="g1b1")
        fc1b = gp.tile([128, NFT], F32, tag="fc1b")
        fc2b = gp.tile([8, D], F32R, tag="fc2b")
        bqk = {p: gp.tile([128, 2 * NDC], F32, tag=f"bqk_{p}", name=f"bqk_{p}")
               for p in BRANCHES}
        consts_loaded = [False]

        def load_consts():
            if consts_loaded[0]:
                return
            consts_loaded[0] = True
            nc.sync.dma_start(ones8[:], dram["ones128"][:].bitcast(F32R))
            nc.sync.dma_start(ones_row[:], dram["ones512"][:].bitcast(F32R))
            nc.vector.memset(eps[:], 1e-5)
            nc.sync.dma_start(gateT[:], dram["gateT"][:].bitcast(F32R))
            nc.sync.dma_start(ginv8[:], dram["ginv8"][:])
            nc.sync.dma_start(expand8[:], dram["expand8"][:].bitcast(F32R))
            nc.sync.dma_start(OB[:], dram["OB"][:].bitcast(F32R))
            nc.sync.dma_start(g1b1[:], dram["g1b1"][:])
            nc.sync.dma_start(fc1b[:], dram["fc1b"][:])
            nc.sync.dma_start(fc2b[:], dram["fc2b"][:].bitcast(F32R))
            for p in BRANCHES:
                nc.sync.dma_start(bqk[p][:], dram[f"bqk_{p}"][:])

        _attention(nc, tc, dram, r128, nx, ones8, gateT, ginv8,
                   expand8, OB, bqk, load_consts)
        _ffn(nc, tc, dram, r128, out_t, nx, ones_col, ones_row, eps, g1b1,
             fc1b, fc2b, gpps)


def _attention(nc, tc, dram, r128, nx, ones8, gateT, ginv8,
                   expand8, OB, bqk, load_consts):
    with tc.tile_pool(name="att", bufs=1) as ap, \
         tc.tile_pool(name="att2", bufs=2) as ap2, \
         tc.tile_pool(name="attoct", bufs=1) as octp, \
         tc.tile_pool(name="attpt", bufs=3) as ptp, \
         tc.tile_pool(name="attsm", bufs=1) as smp, \
         tc.tile_pool(name="attps", bufs=2, space="PSUM") as psg, \
         tc.tile_pool(name="attpsst", bufs=2, space="PSUM") as psst, \
         tc.tile_pool(name="attps1", bufs=1, space="PSUM") as ps1:

        xT = ap.tile([128, NDC, S], F32R, tag="xT")
        xt_loaded = [False]
        mrev = ap.tile([128, NJT, SQ], BF16, tag="mrev")
        mfwd = ap.tile([128, NJT, SQ], BF16, tag="mfwd")
        mask_loaded = [False]

        def _load_masks():
            if not mask_loaded[0]:
                nc.sync.dma_start(mrev[:], dram["mrevT"].rearrange("(j p) q -> p j q", p=128)[:])
                nc.sync.dma_start(mfwd[:], dram["mfwdT"].rearrange("(j p) q -> p j q", p=128)[:])
                mask_loaded[0] = True

        # (branch, kv source name, col offset, mask, K-src override)
        units = [
            ("sa", "xT", 0, None),
            ("mca", "refT", 0, "rev"),
            ("mca", "refT", S, "rev"),
            ("ca", "refT", S, None),
            ("nsa", "refT", S, "fwd"),   # K from ref_last, V from x
            ("msa", "xT", 0, "fwd"),
        ]
        import os as _os
        _nu = int(_os.environ.get("KERNEL_NUM_UNITS", "6"))
        units = units[-_nu:]

        first_op = [True]
        qt_mca = [None]

        for uidx, (p, srcname, coff, mask) in enumerate(units):
            w_sb = ap.tile([128, NDC, 3 * D], F32R, tag="w")
            for c in range(NDC):
                nc.sync.dma_start(w_sb[:, c, 0:D],
                                  r128(f"w_{p}")[:, c, 0:D].bitcast(F32R))
                if not xt_loaded[0]:
                    nc.sync.dma_start(xT[:, c], r128("xT")[:, c].bitcast(F32R))
            xt_loaded[0] = True
            for c in range(NDC):
                nc.sync.dma_start(w_sb[:, c, D:3 * D],
                                  r128(f"w_{p}")[:, c, D:3 * D].bitcast(F32R))
            if srcname == "xT":
                ksrc = xT
                kcoff = 0
            else:
                ksrc = ap2.tile([128, NDC, S], F32R, tag="kvsrc")
                for c in range(NDC):
                    nc.sync.dma_start(
                        ksrc[:, c],
                        dram["refT"].rearrange("(c p) f -> p c f", p=128)
                        [:, c, coff:coff + S].bitcast(F32R))
                kcoff = 0
            vsrc, vcoff = (xT, 0) if p in ("nsa", "msa", "sa") else (ksrc, kcoff)
            load_consts()
            _load_masks()

            # ---- projections ----
            if p == "mca" and qt_mca[0] is not None:
                qt = qt_mca[0]
            else:
                qt = ap.tile([128, NDC, SQ], F32R, tag="qt")
                for t in range(NDC):
                    ps = psg.tile([128, SQ], F32, tag="psgen")
                    for c in range(NDC):
                        nc.tensor.matmul(
                            ps[:], w_sb[:, c, 128 * t:128 * (t + 1)],
                            xT[:, c, 0:SQ], start=(c == 0), stop=(c == NDC - 1))
                    nc.vector.tensor_scalar(qt[:, t], ps[:], bqk[p][:, t:t + 1],
                                            None, ALU.add)
                if p == "mca":
                    qt_mca[0] = qt
            kt = ap.tile([128, NDC, S], F32R, tag="kt")
            for t in range(NDC):
                for s in range(2):
                    ps = psg.tile([128, SQ], F32, tag="psgen")
                    for c in range(NDC):
                        nc.tensor.matmul(
                            ps[:], w_sb[:, c, D + 128 * t:D + 128 * (t + 1)],
                            ksrc[:, c, kcoff + SQ * s:kcoff + SQ * (s + 1)],
                            start=(c == 0), stop=(c == NDC - 1))
                    nc.vector.tensor_scalar(
                        kt[:, t, SQ * s:SQ * (s + 1)], ps[:],
                        bqk[p][:, NDC + t:NDC + t + 1], None, ALU.add)
            js = MCA_JS if mask == "rev" else list(range(NJT))
            va = ap.tile([128, NJT, H, HD + 1], F32R, tag="va")
            for j in js:
                ps = psg.tile([128, SQ], F32, tag="psgen")
                for c in range(NDC):
                    nc.tensor.matmul(
                        ps[:], vsrc[:, c, vcoff + 128 * j:vcoff + 128 * (j + 1)],
                        w_sb[:, c, 2 * D:3 * D],
                        start=(c == 0), stop=(c == NDC - 1))
                nc.vector.tensor_copy(
                    va[:, j, :, 0:HD], ps[:].rearrange("p (h d) -> p h d", h=H))
                nc.vector.tensor_copy(va[:, j, :, HD:HD + 1],
                                      ones8[:, 0:H].unsqueeze(2))

            # ---- attention ----
            rs = smp.tile([H, SQ], F32, tag="rs")
            oct_sb = octp.tile([65, H, SQ], F32R, tag="oct")
            for hp in range(H // 2):
                av2 = ps1.tile([128, 2, SQ], F32, tag="av2")
                t = hp  # kt/qt tile holding this pair (rows 0:64 / 64:128)
                first = True
                for j in js:
                    win = _window(j) if mask else None
                    if mask == "rev":
                        qlo, qhi = win
                    else:
                        qlo, qhi = 0, SQ
                    stp = psst.tile([128, 2, SQ], F32, tag="stps", name="stp")
                    ptt = ptp.tile([128, 2, SQ], F32R, tag="ptt", name="ptt")
                    # two K=64 scores matmuls on distinct PE row groups,
                    # issued back-to-back -> concurrent on HW
                    for hh in range(2):
                        r0 = 64 * hh
                        nc.tensor.matmul(
                            stp[:, hh, qlo:qhi],
                            kt[r0:r0 + 64, t, 128 * j:128 * (j + 1)],
                            qt[r0:r0 + 64, t, qlo:qhi],
                            start=True, stop=True, tile_position=(r0, 0))
                    if qhi - qlo >= 384:     # one fused exp over both banks
                        nc.scalar.activation(ptt[:], stp[:], ACT.Exp)
                    else:
                        for hh in range(2):
                            nc.scalar.activation(ptt[:, hh, qlo:qhi],
                                                 stp[:, hh, qlo:qhi], ACT.Exp)
                    for hh in range(2):
                        if mask == "rev":
                            nc.gpsimd.tensor_mul(
                                ptt[:, hh, qlo:qhi],
                                ptt[:, hh, qlo:qhi].bitcast(F32),
                                mrev[:, j, qlo:qhi])
                        elif mask == "fwd" and win is not None:
                            wl, wh = win
                            nc.gpsimd.tensor_mul(
                                ptt[:, hh, wl:wh], ptt[:, hh, wl:wh].bitcast(F32),
                                mfwd[:, j, wl:wh])
                    for hh in range(2):
                        h = 2 * hp + hh
                        nc.tensor.matmul(
                            av2[0:HD + 1, hh, qlo:qhi], va[:, j, h, :],
                            ptt[:, hh, qlo:qhi],
                            start=first, stop=(j == js[-1]))
                    first = False

                nc.scalar.activation(oct_sb[0:HD, 2 * hp, :], av2[0:HD, 0, :],
                                     ACT.Copy)
                nc.vector.tensor_copy(oct_sb[0:HD, 2 * hp + 1, :],
                                      av2[0:HD, 1, :])
                nc.vector.tensor_copy(oct_sb[64:65, 2 * hp:2 * hp + 2, :],
                                      av2[HD:HD + 1, :, :])
            # gather denominators (lane 64 -> lanes 0..7) and normalize
            nc.sync.dma_start(rs[:], oct_sb[64:65, :, :].bitcast(F32))
            grinv = smp.tile([H, SQ], F32, tag="grinv")
            gr = BRANCHES.index(p)
            nc.vector.tensor_tensor(
                grinv[:], rs[:], ginv8[:, gr * SQ:(gr + 1) * SQ], ALU.mult)
            nc.scalar.activation(grinv[:], grinv[:], ACT.Ln)
            grinvr = smp.tile([H, SQ], F32R, tag="grinvr")
            nc.scalar.activation(grinvr[:], grinv[:], ACT.Exp, scale=-1.0)
            for h in range(H):
                gx = psst.tile([128, SQ], F32, tag="stps")
                nc.tensor.matmul(gx[0:HD, :], expand8[:, h * HD:(h + 1) * HD],
                                 grinvr[:], start=True, stop=True)
                nc.vector.tensor_tensor(
                    oct_sb[0:HD, h, :],
                    oct_sb[0:HD, h, :].bitcast(F32),
                    gx[0:HD, :], ALU.mult)
            # out-projection accumulate into nx
            ow_sb = ap.tile([64, H, D], F32R, tag="ow")
            nc.sync.dma_start(
                ow_sb[:], dram[f"ow_{p}"].rearrange("p (h d) -> p h d", h=H)[:].bitcast(F32R))
            last_u = uidx == len(units) - 1
            for t in range(NDC):
                ps_full = ps1.tile([128, 2, SQ], F32, tag="av2", name="ps_full")
                ps = ps_full[:, 0, :]
                for hc in range(H):
                    nc.tensor.matmul(
                        ps[:], ow_sb[:, hc, 128 * t:128 * (t + 1)],
                        oct_sb[0:HD, hc, :],
                        start=(hc == 0), stop=(hc == H - 1 and not last_u))
                if last_u:
                    nc.tensor.matmul(
                        ps[:], OB[0:5, 128 * t:128 * (t + 1)], gateT[0:5, :],
                        start=False, stop=True)
                if first_op[0]:
                    nc.vector.tensor_copy(nx[:, t], ps[:])
                else:
                    nc.vector.tensor_tensor(nx[:, t], nx[:, t].bitcast(F32), ps[:],
                                            ALU.add)
            first_op[0] = False


def _layernorm(nc, lnp, psg, ones_col, ones_row, eps, src, dst):
    """dst[:, c] = (src[:, c] - mean_D) / sqrt(var_D + eps); src F32R [NDC,128,SQ]."""
    stats = psg.tile([128, SQ], F32, tag="psgen")
    stats2 = psg.tile([128, SQ], F32, tag="psgen")
    sq = lnp.tile([128, NDC, SQ], F32R, tag="sq")
    for c in range(NDC):
        nc.scalar.activation(sq[:, c], src[:, c].bitcast(F32), ACT.Square)
    for c in range(NDC):
        nc.tensor.matmul(stats[0:1, :], ones_col[:, 0:1], src[:, c],
                         start=(c == 0), stop=(c == NDC - 1))
    for c in range(NDC):
        nc.tensor.matmul(stats2[0:1, :], ones_col[:, 0:1], sq[:, c],
                         start=(c == 0), stop=(c == NDC - 1))
    sc = lnp.tile([1, 4 * SQ], F32, tag="lnsc")   # mean | msq | var | rstd
    nc.vector.tensor_scalar(sc[0:1, 0:SQ], stats[0:1, :], 1.0 / D, None, ALU.mult)
    nc.vector.tensor_scalar(sc[0:1, SQ:2 * SQ], stats2[0:1, :], 1.0 / D, None,
                            ALU.mult)
    nc.vector.tensor_tensor(sc[0:1, 2 * SQ:3 * SQ], sc[0:1, 0:SQ],
                            sc[0:1, 0:SQ], ALU.mult)
    nc.vector.tensor_tensor(sc[0:1, 2 * SQ:3 * SQ], sc[0:1, SQ:2 * SQ],
                            sc[0:1, 2 * SQ:3 * SQ], ALU.subtract)
    nc.scalar.activation(sc[0:1, 3 * SQ:4 * SQ], sc[0:1, 2 * SQ:3 * SQ],
                         ACT.Ln, bias=eps[0:1, 0:1])
    scr = lnp.tile([1, 2 * SQ], F32R, tag="lnscr")
    nc.vector.tensor_copy(scr[0:1, 0:SQ], sc[0:1, 0:SQ])
    nc.scalar.activation(scr[0:1, SQ:2 * SQ], sc[0:1, 3 * SQ:4 * SQ],
                         ACT.Exp, scale=-0.5)
    meanx = psg.tile([128, SQ], F32, tag="psgen")
    rstdx = psg.tile([128, SQ], F32, tag="psgen")
    nc.tensor.matmul(meanx[:], ones_row[0:1, 0:128], scr[0:1, 0:SQ],
                     start=True, stop=True)
    nc.tensor.matmul(rstdx[:], ones_row[0:1, 0:128], scr[0:1, SQ:2 * SQ],
                     start=True, stop=True)
    for c in range(NDC):
        t = lnp.tile([128, SQ], F32, tag="lntmp")
        nc.vector.tensor_tensor(t[:], src[:, c].bitcast(F32), meanx[:],
                                ALU.subtract)
        nc.vector.tensor_tensor(dst[:, c], t[:], rstdx[:], ALU.mult)


def _ffn(nc, tc, dram, r128, out_t, nx, ones_col, ones_row, eps, g1b1,
         fc1b, fc2b, gpps):
    with tc.tile_pool(name="ffn", bufs=1) as fp, \
         tc.tile_pool(name="ffnps", bufs=2, space="PSUM") as psg:
        z1 = fp.tile([128, NDC, SQ], F32R, tag="z1")
        _layernorm(nc, fp, psg, ones_col, ones_row, eps, nx, z1)

        fc1 = fp.tile([128, NDC, F], F32R, tag="fc1")
        nc.sync.dma_start(fc1[:], r128("fc1T")[:].bitcast(F32R))
        yT = fp.tile([128, NFT, SQ], F32R, tag="yT")
        for f in range(NFT):
            ps = psg.tile([128, SQ], F32, tag="psffn")
            for c in range(NDC):
                nc.tensor.matmul(ps[:], fc1[:, c, 128 * f:128 * (f + 1)], z1[:, c],
                                 start=(c == 0), stop=(c == NDC - 1))
            nc.scalar.activation(yT[:, f], ps[:], ACT.Gelu, bias=fc1b[:, f:f + 1])
        fc2 = fp.tile([128, NFT, D], F32R, tag="fc2")
        nc.sync.dma_start(fc2[:], dram["fc2T"].rearrange("(c p) d -> p c d", p=128)[:].bitcast(F32R))
        resid = fp.tile([128, NDC, SQ], F32R, tag="resid")
        ps4 = psg.tile([128, NDC, SQ], F32, tag="ps4", name="ps4", bufs=1)
        for f in range(NFT):
            for t in range(NDC):
                nc.tensor.matmul(ps4[:, t], fc2[:, f, 128 * t:128 * (t + 1)],
                                 yT[:, f], start=(f == 0), stop=False)
        for t in range(NDC):
            nc.tensor.matmul(ps4[:, t], fc2b[0:1, 128 * t:128 * (t + 1)],
                             ones_row[:], start=False, stop=True)
            ht = fp.tile([128, SQ], F32, tag="ht")
            nc.vector.tensor_scalar(ht[:], z1[:, t].bitcast(F32), g1b1[:, t:t + 1],
                                    g1b1[:, NDC + t:NDC + t + 1], ALU.mult,
                                    ALU.add)
            nc.vector.tensor_tensor(resid[:, t], ht[:], ps4[:, t], ALU.add)
        z2 = fp.tile([128, NDC, SQ], F32, tag="z2")
        _layernorm(nc, fp, psg, ones_col, ones_row, eps, resid, z2)
        for c in range(NDC):
            nc.sync.dma_start(out_t[128 * c:128 * (c + 1), :], z2[:, c])


# ---------------------------------------------------------------------------
def _split_excess_waits(nc):
    """Walrus caps sync waits (1/inst, 2 on EventSemaphore); peel extras
    onto NoOps inserted before the instruction on the same engine queue."""
    n = 0
    for f in nc.m.functions:
        for bb in f.blocks:
            new = []
            for inst in bb.instructions:
                si = inst.sync_info
                cap = 2 if isinstance(inst, mybir.InstEventSemaphore) else 1
                waits = list(si.on_wait) if si and si.on_wait else []
                if len(waits) > cap:
                    excess, keep = waits[:-cap], waits[-cap:]
                    for i, w in enumerate(excess):
                        nop = mybir.InstNoOp(name=f"{inst.name}_wsplit_{i}",
                                             ins=[], outs=[])
                        nop.engine = inst.engine
                        nop.sync_info = mybir.SyncInfo(on_wait=[w], on_update=[])
                        new.append(nop)
                        n += 1
                    si.on_wait = keep
                    inst.sync_info = si
                new.append(inst)
            bb.instructions = new
    return n


# ---------------------------------------------------------------------------
def _host_prep(inputs):
    x = np.asarray(inputs["x"], np.float32)
    ref = np.asarray(inputs["ref_mca"], np.float32)
    gate = np.asarray(inputs["gate"], np.float32)

    i = np.arange(HGRID)
    near = np.abs(i[:, None] - i[None, :]) <= HALF
    inside = (near[:, None, :, None] & near[None, :, None, :]).reshape(S, S)

    def chunked(v):
        return np.ascontiguousarray(v.reshape(NDC, 128).T)

    per_branch = {}
    OB = np.zeros((8, D), np.float32)
    for bi, p in enumerate(BRANCHES):
        w = np.asarray(inputs[p + "_w"], np.float32)
        b = np.asarray(inputs[p + "_b"], np.float32)
        ow = np.asarray(inputs[p + "_ow"], np.float32)
        ob = np.asarray(inputs[p + "_ob"], np.float32)
        sc = 1.0 / np.sqrt(np.float32(HD))
        wq, wk, wv = w[:D] * sc, w[D:2 * D], w[2 * D:]
        bq, bk, bv = b[:D] * sc, b[D:2 * D], b[2 * D:]
        wT = np.ascontiguousarray(np.concatenate([wq, wk, wv], 0).T)
        bqkv = np.concatenate([chunked(bq), chunked(bk)], 1)
        mult = 2.0 if p == "mca" else 1.0
        OB[bi] = mult * ob + mult * (ow @ bv)
        ow8 = np.ascontiguousarray(
            ow.T.reshape(H, HD, D).transpose(1, 0, 2).reshape(HD, H * D))
        per_branch[p] = (wT, ow8, bqkv)

    g1 = np.asarray(inputs["ln1_g"], np.float32)
    b1 = np.asarray(inputs["ln1_b"], np.float32)
    fc1 = np.asarray(inputs["fc1_w"], np.float32)
    fc1_b = np.asarray(inputs["fc1_b"], np.float32)
    fc2 = np.asarray(inputs["fc2_w"], np.float32)
    fc2_b = np.asarray(inputs["fc2_b"], np.float32)
    fc1T = np.ascontiguousarray((fc1 * g1[None, :]).T)
    fc1bv = np.ascontiguousarray((fc1_b + fc1 @ b1).reshape(NFT, 128).T)
    fc2T = np.ascontiguousarray(fc2.T)
    fc2bv = np.zeros((8, D), np.float32)
    fc2bv[0] = fc2_b
    g1b1 = np.concatenate([chunked(g1), chunked(b1)], 1)

    expand8 = np.zeros((8, 8 * HD), np.float32)
    for h in range(8):
        expand8[h, h * HD:(h + 1) * HD] = 1.0
    in_maps = []
    for core in range(8):
        b_, half = core // 2, core % 2
        q0 = half * SQ
        roll = -q0
        xTr = np.roll(x[b_].T, roll, axis=1)
        refTr = np.concatenate(
            [np.roll(ref[b_, r * S:(r + 1) * S].T, roll, axis=1)
             for r in range(REF)], axis=1)
        insT = np.roll(inside[q0:q0 + SQ, :].T, roll, axis=0)
        mrevT = insT.astype(ml_dtypes.bfloat16)
        mfwdT = (1.0 - insT).astype(ml_dtypes.bfloat16)
        m = {
            "xT": np.ascontiguousarray(xTr),
            "refT": np.ascontiguousarray(refTr),
            "gateT": np.zeros((8, SQ), np.float32),
            "ginvT": np.zeros((8, SQ), np.float32),
            "OB": OB, "mrevT": mrevT, "mfwdT": mfwdT,
            "fc1T": fc1T, "fc1b": fc1bv, "fc2T": fc2T, "fc2b": fc2bv,
            "g1b1": g1b1,
        }
        gslice = gate[b_, q0:q0 + SQ, :].T
        m["gateT"][:5] = gslice
        gi = 1.0 / np.maximum(gslice, 1e-30)          # [5, SQ]
        m["ginv8"] = np.ascontiguousarray(
            np.repeat(gi[None, :, :], 8, axis=0).reshape(8, 5 * SQ))
        m["expand8"] = expand8
        m["ones128"] = np.ones((128, 8), np.float32)
        m["ones512"] = np.ones((1, SQ), np.float32)
        for p in BRANCHES:
            wT, owT, bqkv = per_branch[p]
            m[f"w_{p}"], m[f"ow_{p}"], m[f"bqk_{p}"] = wT, owT, bqkv
        in_maps.append(m)
    return in_maps


_cache = {}


def _get_nc():
    if "nc" not in _cache:
        nc = build_nc()
        _split_excess_waits(nc)
        _cache["nc"] = nc
    return _cache["nc"]


def _get_runner():
    """Compile once; return (fn(in_maps) -> per-core outs, in_names)."""
    if "runner" in _cache:
        return _cache["runner"]
    import jax
    from jax.sharding import Mesh, PartitionSpec
    from jax.experimental.shard_map import shard_map
    import concourse.mybir as mybir_
    from concourse import bass2jax

    nc = _get_nc()
    bass2jax.install_neuronx_cc_hook()
    in_names, out_names, out_avals = [], [], []
    pname = nc.partition_id_tensor.name if nc.partition_id_tensor else None
    for alloc in nc.m.functions[0].allocations:
        if not isinstance(alloc, mybir_.MemoryLocationSet):
            continue
        name = alloc.memorylocations[0].name
        if alloc.kind == "ExternalInput":
            if name != pname:
                in_names.append(name)
        elif alloc.kind == "ExternalOutput":
            out_names.append(name)
            out_avals.append(jax.core.ShapedArray(
                tuple(alloc.tensor_shape), mybir_.dt.np(alloc.dtype)))
    n_params = len(in_names)
    all_names = in_names + out_names + ([pname] if pname else [])

    def _body(*args):
        operands = list(args)
        if pname is not None:
            operands.append(bass2jax.partition_id_tensor())
        return tuple(bass2jax._bass_exec_p.bind(
            *operands, out_avals=tuple(out_avals), in_names=tuple(all_names),
            out_names=tuple(out_names), lowering_input_output_aliases=(),
            sim_require_finite=True, sim_require_nnan=True, nc=nc))

    devices = jax.devices()[:8]
    mesh = Mesh(np.asarray(devices), ("core",))
    nz = len(out_names)
    sharded = jax.jit(shard_map(
        _body, mesh=mesh,
        in_specs=(PartitionSpec("core"),) * (n_params + nz),
        out_specs=(PartitionSpec("core"),) * nz,
        check_rep=False), keep_unused=True)
    zero_shapes = [(8 * a.shape[0], *a.shape[1:]) for a in out_avals]
    zero_dtypes = [a.dtype for a in out_avals]

    def run(in_maps):
        concat_in = [np.concatenate([m[n] for m in in_maps], axis=0)
                     for n in in_names]
        zeros = [np.zeros(s, d) for s, d in zip(zero_shapes, zero_dtypes)]
        outs = sharded(*concat_in, *zeros)
        outs = [np.asarray(o) for o in outs]
        return [
            {n: outs[i].reshape(8, *out_avals[i].shape)[c]
             for i, n in enumerate(out_names)}
            for c in range(8)
        ]

    _cache["runner"] = (run, in_names, sharded, out_avals, out_names)
    return _cache["runner"]


def kernel(**inputs):
    import time as _time
    in_maps = _host_prep(inputs)
    run = _get_runner()[0]
    results = None
    for attempt in range(5):
        try:
            results = run(in_maps)
            break
        except Exception:
            if attempt == 4:
                raise
            # transient device wedge: back off, rebuild the executable
            # (fresh model load) and retry
            _time.sleep(3.0 + 3.0 * attempt)
            try:
                _cache.pop("runner", None)
                import jax as _jax
                _jax.clear_caches()
            except Exception:
                pass
            run = _get_runner()[0]
    
    g2 = np.asarray(inputs["ln2_g"], np.float32)
    b2 = np.asarray(inputs["ln2_b"], np.float32)
    out = np.empty((B, S, D), np.float32)
    for core in range(8):
        b_, half = core // 2, core % 2
        out[b_, half * SQ:(half + 1) * SQ] = results[core]["z2T"].T
    return (out * g2[None, None, :] + b2[None, None, :]).astype(np.float32)


if __name__ == "__main__":
    nc = build_nc()
    n_inst = sum(len(bb.instructions) for f in nc.m.functions for bb in f.blocks)
    print("built ok, insts:", n_inst)
    print("wait splits:", _split_excess_waits(nc))


# revision 32
# speedup vs baseline: 26.8312x; 1.0613x over previous
"""Trainium2 Bass kernel for nn_EncoderLayer_2035814498815 (sparse_attention).

Sharding: 8 cores = (batch sample b in 0..3) x (query half in 0..1).
Zero collectives: each core computes 512 query rows of sample b through the
whole layer (5 MHA branches + gating + LN1 + FFN + LN2); host concatenates.

One SPMD program for all 8 cores: the host rotates the KEY order of every
KV source (and the masks) by 512*half per core, which makes the 7x7-window
block-sparsity geometry identical across cores (attention is permutation-
invariant over keys). Queries always live at rotated columns [0, 512).

On-core layout is transposed ([D, seq]):
- scores come out as ST = K @ Q^T  [Sk_p, Sq_f] (contraction over head dim)
- exp needs no max-subtraction (|scores| ~ 0.2; 1/sqrt(hd) folded into wq)
- A@V uses V as stationary operand augmented with a ones column per head,
  so PSUM row 64 accumulates softmax denominators for free
- denominators are gathered per unit into an [8, 512] tile by tiny DMAs,
  reciprocals run as Ln->Exp(-x) on ACT (same table set as the attention
  exp); gate/rowsum fused via host-sent 1/gate
- out-projection of all units accumulates into one new_x tile; the
  out-proj bias + V-bias terms enter as a host-precomputed rank-5 matmul
  OB^T @ gate^T; LN affine folds into fc1 (g1,b1) and the host (g2,b2).

All matmuls run as float32r (~1.6e-4 rel err, full PE rate at N>=256).
"""

import sys

sys.path.insert(0, "/opt/trn_rl_repo")

import numpy as np
import ml_dtypes

import concourse.bass as bass
import concourse.mybir as mybir
import concourse.tile as tile
from concourse.bass_utils import run_bass_kernel_spmd

F32 = mybir.dt.float32
F32R = mybir.dt.float32r
BF16 = mybir.dt.bfloat16
ACT = mybir.ActivationFunctionType
ALU = mybir.AluOpType

B, S, D, H, HD, F, REF = 4, 1024, 512, 8, 64, 2048, 2
HGRID, HALF = 32, 3
SQ = 512
NDC = D // 128     # 4
NJT = S // 128     # 8
NFT = F // 128     # 16
BRANCHES = ["mca", "ca", "msa", "nsa", "sa"]

# window of local query columns whose 7x7 grid window can intersect rotated
# key tile j  (rotated coords: local i1' in [0,16), key j1' = 4j..4j+3)
def _window(j):
    if j <= 4:
        return (max(0, 4 * j - 3) * 32, min(16, 4 * j + 7) * 32)
    if j == 7:
        return (0, 96)          # wrap-around block (real only on half==1)
    return None


MCA_JS = [j for j in range(NJT) if _window(j) is not None]


def build_nc():
    nc = bass.Bass(trn_type="TRN2")
    dram = {}

    def din(name, shape, dt=F32):
        dram[name] = nc.dram_tensor(name, shape, dt, kind="ExternalInput")

    din("xT", [D, S])
    din("refT", [D, REF * S])
    for p in BRANCHES:
        din(f"w_{p}", [D, 3 * D])
        din(f"ow_{p}", [D, D])
        din(f"bqk_{p}", [128, 2 * NDC])
    din("gateT", [8, SQ])
    din("ginv8", [8, 5 * SQ])
    din("expand8", [8, 8 * HD])
    din("ones128", [128, 8])
    din("ones512", [1, SQ])
    din("OB", [8, D])
    din("mrevT", [S, SQ], BF16)
    din("mfwdT", [S, SQ], BF16)
    din("fc1T", [D, F])
    din("fc1b", [128, NFT])
    din("fc2T", [F, D])
    din("fc2b", [8, D])
    din("g1b1", [128, 2 * NDC])
    out_t = nc.dram_tensor("z2T", [D, SQ], F32, kind="ExternalOutput")

    with tile.TileContext(nc) as tc:
        _emit(nc, tc, dram, out_t)
    return nc


def _emit(nc, tc, dram, out_t):
    def r128(name):
        return dram[name].rearrange("(c p) f -> p c f", p=128)

    # ---------------- global pools (live whole kernel) ----------------
    with tc.tile_pool(name="glob", bufs=1) as gp, \
         tc.tile_pool(name="globps", bufs=1, space="PSUM") as gpps:

        nx = gp.tile([128, NDC, SQ], F32R, tag="nx")
        ones8 = gp.tile([128, 8], F32R, tag="ones8")
        ones_col = ones8
        ones_row = gp.tile([1, SQ], F32R, tag="ones_row")
        eps = gp.tile([1, 1], F32, tag="eps")
        gateT = gp.tile([8, SQ], F32R, tag="gateT")
        ginv8 = gp.tile([8, 5 * SQ], F32, tag="ginv8")
        expand8 = gp.tile([8, 8 * HD], F32R, tag="expand8")
        OB = gp.tile([8, D], F32R, tag="OB")
        g1b1 = gp.tile([128, 2 * NDC], F32, tag="g1b1")
        fc1b = gp.tile([128, NFT], F32, tag="fc1b")
        fc2b = gp.tile([8, D], F32R, tag="fc2b")
        bqk = {p: gp.tile([128, 2 * NDC], F32, tag=f"bqk_{p}", name=f"bqk_{p}")
               for p in BRANCHES}
        consts_loaded = [False]

        def load_consts():
            if consts_loaded[0]:
                return
            consts_loaded[0] = True
            nc.sync.dma_start(ones8[:], dram["ones128"][:].bitcast(F32R))
            nc.sync.dma_start(ones_row[:], dram["ones512"][:].bitcast(F32R))
            nc.vector.memset(eps[:], 1e-5)
            nc.sync.dma_start(gateT[:], dram["gateT"][:].bitcast(F32R))
            nc.sync.dma_start(ginv8[:], dram["ginv8"][:])
            nc.sync.dma_start(expand8[:], dram["expand8"][:].bitcast(F32R))
            nc.sync.dma_start(OB[:], dram["OB"][:].bitcast(F32R))
            nc.sync.dma_start(g1b1[:], dram["g1b1"][:])
            nc.sync.dma_start(fc1b[:], dram["fc1b"][:])
            nc.sync.dma_start(fc2b[:], dram["fc2b"][:].bitcast(F32R))
            for p in BRANCHES:
                nc.sync.dma_start(bqk[p][:], dram[f"bqk_{p}"][:])

        _attention(nc, tc, dram, r128, nx, ones8, gateT, ginv8,
                   expand8, OB, bqk, load_consts)
        _ffn(nc, tc, dram, r128, out_t, nx, ones_col, ones_row, eps, g1b1,
             fc1b, fc2b, gpps)


def _attention(nc, tc, dram, r128, nx, ones8, gateT, ginv8,
                   expand8, OB, bqk, load_consts):
    with tc.tile_pool(name="att", bufs=1) as ap, \
         tc.tile_pool(name="att2", bufs=2) as ap2, \
         tc.tile_pool(name="attoct", bufs=1) as octp, \
         tc.tile_pool(name="attpt", bufs=3) as ptp, \
         tc.tile_pool(name="attsm", bufs=1) as smp, \
         tc.tile_pool(name="attps", bufs=2, space="PSUM") as psg, \
         tc.tile_pool(name="attpsst", bufs=2, space="PSUM") as psst, \
         tc.tile_pool(name="attps1", bufs=1, space="PSUM") as ps1:

        xT = ap.tile([128, NDC, S], F32R, tag="xT")
        xt_loaded = [False]
        mrev = ap.tile([128, NJT, SQ], BF16, tag="mrev")
        mfwd = ap.tile([128, NJT, SQ], BF16, tag="mfwd")
        mask_loaded = [False]

        def _load_masks():
            if not mask_loaded[0]:
                nc.sync.dma_start(mrev[:], dram["mrevT"].rearrange("(j p) q -> p j q", p=128)[:])
                nc.sync.dma_start(mfwd[:], dram["mfwdT"].rearrange("(j p) q -> p j q", p=128)[:])
                mask_loaded[0] = True

        # (branch, kv source name, col offset, mask, K-src override)
        units = [
            ("sa", "xT", 0, None),
            ("mca", "refT", 0, "rev"),
            ("mca", "refT", S, "rev"),
            ("ca", "refT", S, None),
            ("nsa", "refT", S, "fwd"),   # K from ref_last, V from x
            ("msa", "xT", 0, "fwd"),
        ]
        import os as _os
        _nu = int(_os.environ.get("KERNEL_NUM_UNITS", "6"))
        units = units[-_nu:]

        first_op = [True]
        qt_mca = [None]

        for uidx, (p, srcname, coff, mask) in enumerate(units):
            w_sb = ap.tile([128, NDC, 3 * D], F32R, tag="w")
            for c in range(NDC):
                nc.sync.dma_start(w_sb[:, c, 0:D],
                                  r128(f"w_{p}")[:, c, 0:D].bitcast(F32R))
                if not xt_loaded[0]:
                    nc.sync.dma_start(xT[:, c], r128("xT")[:, c].bitcast(F32R))
            xt_loaded[0] = True
            for c in range(NDC):
                nc.sync.dma_start(w_sb[:, c, D:3 * D],
                                  r128(f"w_{p}")[:, c, D:3 * D].bitcast(F32R))
            if srcname == "xT":
                ksrc = xT
                kcoff = 0
            else:
                ksrc = ap2.tile([128, NDC, S], F32R, tag="kvsrc")
                for c in range(NDC):
                    nc.sync.dma_start(
                        ksrc[:, c],
                        dram["refT"].rearrange("(c p) f -> p c f", p=128)
                        [:, c, coff:coff + S].bitcast(F32R))
                kcoff = 0
            vsrc, vcoff = (xT, 0) if p in ("nsa", "msa", "sa") else (ksrc, kcoff)
            load_consts()
            _load_masks()

            # ---- projections ----
            if p == "mca" and qt_mca[0] is not None:
                qt = qt_mca[0]
            else:
                qt = ap.tile([128, NDC, SQ], F32R, tag="qt")
                for t in range(NDC):
                    ps = psg.tile([128, SQ], F32, tag="psgen")
                    for c in range(NDC):
                        nc.tensor.matmul(
                            ps[:], w_sb[:, c, 128 * t:128 * (t + 1)],
                            xT[:, c, 0:SQ], start=(c == 0), stop=(c == NDC - 1))
                    nc.vector.tensor_scalar(qt[:, t], ps[:], bqk[p][:, t:t + 1],
                                            None, ALU.add)
                if p == "mca":
                    qt_mca[0] = qt
            kt = ap.tile([128, NDC, S], F32R, tag="kt")
            for t in range(NDC):
                for s in range(2):
                    ps = psg.tile([128, SQ], F32, tag="psgen")
                    for c in range(NDC):
                        nc.tensor.matmul(
                            ps[:], w_sb[:, c, D + 128 * t:D + 128 * (t + 1)],
                            ksrc[:, c, kcoff + SQ * s:kcoff + SQ * (s + 1)],
                            start=(c == 0), stop=(c == NDC - 1))
                    nc.vector.tensor_scalar(
                        kt[:, t, SQ * s:SQ * (s + 1)], ps[:],
                        bqk[p][:, NDC + t:NDC + t + 1], None, ALU.add)
            js = MCA_JS if mask == "rev" else list(range(NJT))
            va = ap.tile([128, NJT, H, HD + 1], F32R, tag="va")
            for j in js:
                ps = psg.tile([128, SQ], F32, tag="psgen")
                for c in range(NDC):
                    nc.tensor.matmul(
                        ps[:], vsrc[:, c, vcoff + 128 * j:vcoff + 128 * (j + 1)],
                        w_sb[:, c, 2 * D:3 * D],
                        start=(c == 0), stop=(c == NDC - 1))
                nc.vector.tensor_copy(
                    va[:, j, :, 0:HD], ps[:].rearrange("p (h d) -> p h d", h=H))
                nc.vector.tensor_copy(va[:, j, :, HD:HD + 1],
                                      ones8[:, 0:H].unsqueeze(2))

            # ---- attention ----
            rs = smp.tile([H, SQ], F32, tag="rs")
            oct_sb = octp.tile([128, NDC, SQ], F32R, tag="oct")
            for hp in range(H // 2):
                av2 = ps1.tile([128, 2, SQ], F32, tag="av2")
                t = hp  # kt/qt tile holding this pair (rows 0:64 / 64:128)
                first = True
                for j in js:
                    win = _window(j) if mask else None
                    if mask == "rev":
                        qlo, qhi = win
                    else:
                        qlo, qhi = 0, SQ
                    stp = psst.tile([128, 2, SQ], F32, tag="stps", name="stp")
                    ptt = ptp.tile([128, 2, SQ], F32R, tag="ptt", name="ptt")
                    # two K=64 scores matmuls on distinct PE row groups,
                    # issued back-to-back -> concurrent on HW
                    for hh in range(2):
                        r0 = 64 * hh
                        nc.tensor.matmul(
                            stp[:, hh, qlo:qhi],
                            kt[r0:r0 + 64, t, 128 * j:128 * (j + 1)],
                            qt[r0:r0 + 64, t, qlo:qhi],
                            start=True, stop=True, tile_position=(r0, 0))
                    if qhi - qlo >= 384:     # one fused exp over both banks
                        nc.scalar.activation(ptt[:], stp[:], ACT.Exp)
                    else:
                        for hh in range(2):
                            nc.scalar.activation(ptt[:, hh, qlo:qhi],
                                                 stp[:, hh, qlo:qhi], ACT.Exp)
                    for hh in range(2):
                        if mask == "rev":
                            nc.gpsimd.tensor_mul(
                                ptt[:, hh, qlo:qhi],
                                ptt[:, hh, qlo:qhi].bitcast(F32),
                                mrev[:, j, qlo:qhi])
                        elif mask == "fwd" and win is not None:
                            wl, wh = win
                            nc.gpsimd.tensor_mul(
                                ptt[:, hh, wl:wh], ptt[:, hh, wl:wh].bitcast(F32),
                                mfwd[:, j, wl:wh])
                    for hh in range(2):
                        h = 2 * hp + hh
                        nc.tensor.matmul(
                            av2[0:HD + 1, hh, qlo:qhi], va[:, j, h, :],
                            ptt[:, hh, qlo:qhi],
                            start=first, stop=(j == js[-1]))
                    first = False

                otmp = ptp.tile([65, 2, SQ], F32R, tag="otmp", name="otmp",
                                bufs=2)
                nc.scalar.activation(oct_sb[0:HD, hp, :], av2[0:HD, 0, :],
                                     ACT.Copy)
                nc.vector.tensor_copy(otmp[0:HD, 1, :], av2[0:HD, 1, :])
                nc.vector.tensor_copy(otmp[64:65, 0:2, :],
                                      av2[HD:HD + 1, :, :])
                # cross-lane moves via SBUF->SBUF DMA: odd head to rows
                # 64:128 (enables K=128 out-proj), denominators to rs rows
                nc.sync.dma_start(oct_sb[64:128, hp, :],
                                  otmp[0:HD, 1, :])
                nc.sync.dma_start(rs[2 * hp:2 * hp + 2, :],
                                  otmp[64:65, 0:2, :].bitcast(F32))
            # normalize
            grinv = smp.tile([H, SQ], F32, tag="grinv")
            gr = BRANCHES.index(p)
            nc.vector.tensor_tensor(
                grinv[:], rs[:], ginv8[:, gr * SQ:(gr + 1) * SQ], ALU.mult)
            nc.scalar.activation(grinv[:], grinv[:], ACT.Ln)
            grinvr = smp.tile([H, SQ], F32R, tag="grinvr")
            nc.scalar.activation(grinvr[:], grinv[:], ACT.Exp, scale=-1.0)
            for hp in range(H // 2):
                gx = psst.tile([128, SQ], F32, tag="stps")
                nc.tensor.matmul(gx[:], expand8[:, hp * 128:(hp + 1) * 128],
                                 grinvr[:], start=True, stop=True)
                nc.vector.tensor_tensor(
                    oct_sb[:, hp, :], oct_sb[:, hp, :].bitcast(F32),
                    gx[:], ALU.mult)
            # out-projection accumulate into nx
            ow_sb = ap.tile([128, NDC, D], F32R, tag="ow")
            nc.sync.dma_start(
                ow_sb[:], r128(f"ow_{p}")[:].bitcast(F32R))
            last_u = uidx == len(units) - 1
            for t in range(NDC):
                ps_full = ps1.tile([128, 2, SQ], F32, tag="av2", name="ps_full")
                ps = ps_full[:, 0, :]
                for c in range(NDC):
                    nc.tensor.matmul(
                        ps[:], ow_sb[:, c, 128 * t:128 * (t + 1)],
                        oct_sb[:, c, :],
                        start=(c == 0), stop=(c == NDC - 1 and not last_u))
                if last_u:
                    nc.tensor.matmul(
                        ps[:], OB[0:5, 128 * t:128 * (t + 1)], gateT[0:5, :],
                        start=False, stop=True)
                if first_op[0]:
                    nc.vector.tensor_copy(nx[:, t], ps[:])
                else:
                    nc.vector.tensor_tensor(nx[:, t], nx[:, t].bitcast(F32), ps[:],
                                            ALU.add)
            first_op[0] = False


def _layernorm(nc, lnp, psg, ones_col, ones_row, eps, src, dst):
    """dst[:, c] = (src[:, c] - mean_D) / sqrt(var_D + eps); src F32R [NDC,128,SQ]."""
    stats = psg.tile([128, SQ], F32, tag="psgen")
    stats2 = psg.tile([128, SQ], F32, tag="psgen")
    sq = lnp.tile([128, NDC, SQ], F32R, tag="sq")
    for c in range(NDC):
        nc.scalar.activation(sq[:, c], src[:, c].bitcast(F32), ACT.Square)
    for c in range(NDC):
        nc.tensor.matmul(stats[0:1, :], ones_col[:, 0:1], src[:, c],
                         start=(c == 0), stop=(c == NDC - 1))
    for c in range(NDC):
        nc.tensor.matmul(stats2[0:1, :], ones_col[:, 0:1], sq[:, c],
                         start=(c == 0), stop=(c == NDC - 1))
    sc = lnp.tile([1, 4 * SQ], F32, tag="lnsc")   # mean | msq | var | rstd
    nc.vector.tensor_scalar(sc[0:1, 0:SQ], stats[0:1, :], 1.0 / D, None, ALU.mult)
    nc.vector.tensor_scalar(sc[0:1, SQ:2 * SQ], stats2[0:1, :], 1.0 / D, None,
                            ALU.mult)
    nc.vector.tensor_tensor(sc[0:1, 2 * SQ:3 * SQ], sc[0:1, 0:SQ],
                            sc[0:1, 0:SQ], ALU.mult)
    nc.vector.tensor_tensor(sc[0:1, 2 * SQ:3 * SQ], sc[0:1, SQ:2 * SQ],
                            sc[0:1, 2 * SQ:3 * SQ], ALU.subtract)
    nc.scalar.activation(sc[0:1, 3 * SQ:4 * SQ], sc[0:1, 2 * SQ:3 * SQ],
                         ACT.Ln, bias=eps[0:1, 0:1])
    scr = lnp.tile([1, 2 * SQ], F32R, tag="lnscr")
    nc.vector.tensor_copy(scr[0:1, 0:SQ], sc[0:1, 0:SQ])
    nc.scalar.activation(scr[0:1, SQ:2 * SQ], sc[0:1, 3 * SQ:4 * SQ],
                         ACT.Exp, scale=-0.5)
    meanx = psg.tile([128, SQ], F32, tag="psgen")
    rstdx = psg.tile([128, SQ], F32, tag="psgen")
    nc.tensor.matmul(meanx[:], ones_row[0:1, 0:128], scr[0:1, 0:SQ],
                     start=True, stop=True)
    nc.tensor.matmul(rstdx[:], ones_row[0:1, 0:128], scr[0:1, SQ:2 * SQ],
                     start=True, stop=True)
    for c in range(NDC):
        t = lnp.tile([128, SQ], F32, tag="lntmp")
        nc.vector.tensor_tensor(t[:], src[:, c].bitcast(F32), meanx[:],
                                ALU.subtract)
        nc.vector.tensor_tensor(dst[:, c], t[:], rstdx[:], ALU.mult)


def _ffn(nc, tc, dram, r128, out_t, nx, ones_col, ones_row, eps, g1b1,
         fc1b, fc2b, gpps):
    with tc.tile_pool(name="ffn", bufs=1) as fp, \
         tc.tile_pool(name="ffnps", bufs=2, space="PSUM") as psg:
        z1 = fp.tile([128, NDC, SQ], F32R, tag="z1")
        _layernorm(nc, fp, psg, ones_col, ones_row, eps, nx, z1)

        fc1 = fp.tile([128, NDC, F], F32R, tag="fc1")
        nc.sync.dma_start(fc1[:], r128("fc1T")[:].bitcast(F32R))
        yT = fp.tile([128, NFT, SQ], F32R, tag="yT")
        for f in range(NFT):
            ps = psg.tile([128, SQ], F32, tag="psffn")
            for c in range(NDC):
                nc.tensor.matmul(ps[:], fc1[:, c, 128 * f:128 * (f + 1)], z1[:, c],
                                 start=(c == 0), stop=(c == NDC - 1))
            nc.scalar.activation(yT[:, f], ps[:], ACT.Gelu, bias=fc1b[:, f:f + 1])
        fc2 = fp.tile([128, NFT, D], F32R, tag="fc2")
        nc.sync.dma_start(fc2[:], dram["fc2T"].rearrange("(c p) d -> p c d", p=128)[:].bitcast(F32R))
        resid = fp.tile([128, NDC, SQ], F32R, tag="resid")
        ps4 = psg.tile([128, NDC, SQ], F32, tag="ps4", name="ps4", bufs=1)
        for f in range(NFT):
            for t in range(NDC):
                nc.tensor.matmul(ps4[:, t], fc2[:, f, 128 * t:128 * (t + 1)],
                                 yT[:, f], start=(f == 0), stop=False)
        for t in range(NDC):
            nc.tensor.matmul(ps4[:, t], fc2b[0:1, 128 * t:128 * (t + 1)],
                             ones_row[:], start=False, stop=True)
            ht = fp.tile([128, SQ], F32, tag="ht")
            nc.vector.tensor_scalar(ht[:], z1[:, t].bitcast(F32), g1b1[:, t:t + 1],
                                    g1b1[:, NDC + t:NDC + t + 1], ALU.mult,
                                    ALU.add)
            nc.vector.tensor_tensor(resid[:, t], ht[:], ps4[:, t], ALU.add)
        z2 = fp.tile([128, NDC, SQ], F32, tag="z2")
        _layernorm(nc, fp, psg, ones_col, ones_row, eps, resid, z2)
        for c in range(NDC):
            nc.sync.dma_start(out_t[128 * c:128 * (c + 1), :], z2[:, c])


# ---------------------------------------------------------------------------
def _split_excess_waits(nc):
    """Walrus caps sync waits (1/inst, 2 on EventSemaphore); peel extras
    onto NoOps inserted before the instruction on the same engine queue."""
    n = 0
    for f in nc.m.functions:
        for bb in f.blocks:
            new = []
            for inst in bb.instructions:
                si = inst.sync_info
                cap = 2 if isinstance(inst, mybir.InstEventSemaphore) else 1
                waits = list(si.on_wait) if si and si.on_wait else []
                if len(waits) > cap:
                    excess, keep = waits[:-cap], waits[-cap:]
                    for i, w in enumerate(excess):
                        nop = mybir.InstNoOp(name=f"{inst.name}_wsplit_{i}",
                                             ins=[], outs=[])
                        nop.engine = inst.engine
                        nop.sync_info = mybir.SyncInfo(on_wait=[w], on_update=[])
                        new.append(nop)
                        n += 1
                    si.on_wait = keep
                    inst.sync_info = si
                new.append(inst)
            bb.instructions = new
    return n


# ---------------------------------------------------------------------------
def _host_prep(inputs):
    x = np.asarray(inputs["x"], np.float32)
    ref = np.asarray(inputs["ref_mca"], np.float32)
    gate = np.asarray(inputs["gate"], np.float32)

    i = np.arange(HGRID)
    near = np.abs(i[:, None] - i[None, :]) <= HALF
    inside = (near[:, None, :, None] & near[None, :, None, :]).reshape(S, S)

    def chunked(v):
        return np.ascontiguousarray(v.reshape(NDC, 128).T)

    per_branch = {}
    OB = np.zeros((8, D), np.float32)
    for bi, p in enumerate(BRANCHES):
        w = np.asarray(inputs[p + "_w"], np.float32)
        b = np.asarray(inputs[p + "_b"], np.float32)
        ow = np.asarray(inputs[p + "_ow"], np.float32)
        ob = np.asarray(inputs[p + "_ob"], np.float32)
        sc = 1.0 / np.sqrt(np.float32(HD))
        wq, wk, wv = w[:D] * sc, w[D:2 * D], w[2 * D:]
        bq, bk, bv = b[:D] * sc, b[D:2 * D], b[2 * D:]
        wT = np.ascontiguousarray(np.concatenate([wq, wk, wv], 0).T)
        bqkv = np.concatenate([chunked(bq), chunked(bk)], 1)
        mult = 2.0 if p == "mca" else 1.0
        OB[bi] = mult * ob + mult * (ow @ bv)
        per_branch[p] = (wT, np.ascontiguousarray(ow.T), bqkv)

    g1 = np.asarray(inputs["ln1_g"], np.float32)
    b1 = np.asarray(inputs["ln1_b"], np.float32)
    fc1 = np.asarray(inputs["fc1_w"], np.float32)
    fc1_b = np.asarray(inputs["fc1_b"], np.float32)
    fc2 = np.asarray(inputs["fc2_w"], np.float32)
    fc2_b = np.asarray(inputs["fc2_b"], np.float32)
    fc1T = np.ascontiguousarray((fc1 * g1[None, :]).T)
    fc1bv = np.ascontiguousarray((fc1_b + fc1 @ b1).reshape(NFT, 128).T)
    fc2T = np.ascontiguousarray(fc2.T)
    fc2bv = np.zeros((8, D), np.float32)
    fc2bv[0] = fc2_b
    g1b1 = np.concatenate([chunked(g1), chunked(b1)], 1)

    expand8 = np.zeros((8, 8 * HD), np.float32)
    for hp in range(4):
        expand8[2 * hp, hp * 128:hp * 128 + HD] = 1.0
        expand8[2 * hp + 1, hp * 128 + HD:hp * 128 + 128] = 1.0
    in_maps = []
    for core in range(8):
        b_, half = core // 2, core % 2
        q0 = half * SQ
        roll = -q0
        xTr = np.roll(x[b_].T, roll, axis=1)
        refTr = np.concatenate(
            [np.roll(ref[b_, r * S:(r + 1) * S].T, roll, axis=1)
             for r in range(REF)], axis=1)
        insT = np.roll(inside[q0:q0 + SQ, :].T, roll, axis=0)
        mrevT = insT.astype(ml_dtypes.bfloat16)
        mfwdT = (1.0 - insT).astype(ml_dtypes.bfloat16)
        m = {
            "xT": np.ascontiguousarray(xTr),
            "refT": np.ascontiguousarray(refTr),
            "gateT": np.zeros((8, SQ), np.float32),
            "ginvT": np.zeros((8, SQ), np.float32),
            "OB": OB, "mrevT": mrevT, "mfwdT": mfwdT,
            "fc1T": fc1T, "fc1b": fc1bv, "fc2T": fc2T, "fc2b": fc2bv,
            "g1b1": g1b1,
        }
        gslice = gate[b_, q0:q0 + SQ, :].T
        m["gateT"][:5] = gslice
        gi = 1.0 / np.maximum(gslice, 1e-30)          # [5, SQ]
        m["ginv8"] = np.ascontiguousarray(
            np.repeat(gi[None, :, :], 8, axis=0).reshape(8, 5 * SQ))
        m["expand8"] = expand8
        m["ones128"] = np.ones((128, 8), np.float32)
        m["ones512"] = np.ones((1, SQ), np.float32)
        for p in BRANCHES:
            wT, owT, bqkv = per_branch[p]
            m[f"w_{p}"], m[f"ow_{p}"], m[f"bqk_{p}"] = wT, owT, bqkv
        in_maps.append(m)
    return in_maps


_cache = {}


def _get_nc():
    if "nc" not in _cache:
        nc = build_nc()
        _split_excess_waits(nc)
        _cache["nc"] = nc
    return _cache["nc"]


def _get_runner():
    """Compile once; return (fn(in_maps) -> per-core outs, in_names)."""
    if "runner" in _cache:
        return _cache["runner"]
    import jax
    from jax.sharding import Mesh, PartitionSpec
    from jax.experimental.shard_map import shard_map
    import concourse.mybir as mybir_
    from concourse import bass2jax

    nc = _get_nc()
    bass2jax.install_neuronx_cc_hook()
    in_names, out_names, out_avals = [], [], []
    pname = nc.partition_id_tensor.name if nc.partition_id_tensor else None
    for alloc in nc.m.functions[0].allocations:
        if not isinstance(alloc, mybir_.MemoryLocationSet):
            continue
        name = alloc.memorylocations[0].name
        if alloc.kind == "ExternalInput":
            if name != pname:
                in_names.append(name)
        elif alloc.kind == "ExternalOutput":
            out_names.append(name)
            out_avals.append(jax.core.ShapedArray(
                tuple(alloc.tensor_shape), mybir_.dt.np(alloc.dtype)))
    n_params = len(in_names)
    all_names = in_names + out_names + ([pname] if pname else [])

    def _body(*args):
        operands = list(args)
        if pname is not None:
            operands.append(bass2jax.partition_id_tensor())
        return tuple(bass2jax._bass_exec_p.bind(
            *operands, out_avals=tuple(out_avals), in_names=tuple(all_names),
            out_names=tuple(out_names), lowering_input_output_aliases=(),
            sim_require_finite=True, sim_require_nnan=True, nc=nc))

    devices = jax.devices()[:8]
    mesh = Mesh(np.asarray(devices), ("core",))
    nz = len(out_names)
    sharded = jax.jit(shard_map(
        _body, mesh=mesh,
        in_specs=(PartitionSpec("core"),) * (n_params + nz),
        out_specs=(PartitionSpec("core"),) * nz,
        check_rep=False), keep_unused=True)
    zero_shapes = [(8 * a.shape[0], *a.shape[1:]) for a in out_avals]
    zero_dtypes = [a.dtype for a in out_avals]

    def run(in_maps):
        concat_in = [np.concatenate([m[n] for m in in_maps], axis=0)
                     for n in in_names]
        zeros = [np.zeros(s, d) for s, d in zip(zero_shapes, zero_dtypes)]
        outs = sharded(*concat_in, *zeros)
        outs = [np.asarray(o) for o in outs]
        return [
            {n: outs[i].reshape(8, *out_avals[i].shape)[c]
             for i, n in enumerate(out_names)}
            for c in range(8)
        ]

    _cache["runner"] = (run, in_names, sharded, out_avals, out_names)
    return _cache["runner"]


def kernel(**inputs):
    import time as _time
    in_maps = _host_prep(inputs)
    run = _get_runner()[0]
    results = None
    for attempt in range(5):
        try:
            results = run(in_maps)
            break
        except Exception:
            if attempt == 4:
                raise
            # transient device wedge: back off, rebuild the executable
            # (fresh model load) and retry
            _time.sleep(3.0 + 3.0 * attempt)
            try:
                _cache.pop("runner", None)
                import jax as _jax
                _jax.clear_caches()
            except Exception:
                pass
            run = _get_runner()[0]
    
    g2 = np.asarray(inputs["ln2_g"], np.float32)
    b2 = np.asarray(inputs["ln2_b"], np.float32)
    out = np.empty((B, S, D), np.float32)
    for core in range(8):
        b_, half = core // 2, core % 2
        out[b_, half * SQ:(half + 1) * SQ] = results[core]["z2T"].T
    return (out * g2[None, None, :] + b2[None, None, :]).astype(np.float32)


if __name__ == "__main__":
    nc = build_nc()
    n_inst = sum(len(bb.instructions) for f in nc.m.functions for bb in f.blocks)
    print("built ok, insts:", n_inst)
    print("wait splits:", _split_excess_waits(nc))
